# revision 10
# baseline (speedup 1.0000x reference)
"""Trainium2 Bass kernel for causal top-K (K=8) similarity message passing.

Math per batch b (reference):
  gate  = sigmoid(x @ w_gate + b_gate)                      (B,T)
  S     = x @ x^T, causal-masked to NEG=-1e30               (B,T,T)
  top-8 per row -> selected neighbour set, count=min(i+1,8)
  msg   = mean of selected x rows
  blend = mix*x + (1-mix)*msg
  out   = gate * gelu(blend*gain + bias) * (softplus(log_scale)+0.01)

Sharding: 8 cores = 4 batches x 2 query-parity shards. Core c handles
batch b=c>>1, parity p=c&1, processing query tiles Qg = 2t+p (t=0..T/256-1)
of 128 rows each. Every core runs a uniform program; all parity
dependence is carried in per-core input data (masks / precomputed tables).

Numerics: scores need enough precision that the top-8 *selection* matches
fp32 (rank-8/9 gaps are O(1) while |score| is O(100); bf16 or f32r alone
flips a few % of rows and blows the 2e-2 budget). x^T is therefore split
host-side into bf16 hi+lo halves and S accumulated as
hi*hi + hi*lo + lo*hi in PSUM -- fp32-grade scores at 3 bf16 matmul
passes (vs 4 passes for native fp32 PE). The aggregation path
(sel^T @ x*gain) is plain bf16: msg is a mean of <=8 unit-scale rows, so
bf16 rounding lands ~1e-3 relative, far inside the 2e-2 budget.

Per query tile t (Lc = 2t+2 key chunks of 128; one wasted fully-masked
chunk for p=0 so both parities run the identical program):
  scores  = sum of 3 split matmuls (PE, bf16) -> PSUM -> SBUF + causal mask
  v8      = max8(scores), tau = v8[:,7]      (DVE top-8 instruction)
  sel     = scores >= tau (0/1), fixed up for tile 0; diagonal gets
            mix*count/(1-mix) added so the blend's mix*x term rides the
            aggregation matmul
  msg     = sel^T-transposed chunks @ (x*gain bf16) accumulated in PSUM,
            plus a rank-1 matmul adding bias*count/(1-mix)
  z       = msg * (1-mix)/count   (per-row scale during PSUM->SBUF copy)
  out     = sigmoid(gate_lin)*scale * gelu(z)

The whole per-core computation sits inside a tc.For_i whose trip count is
read at runtime from the tiny `iters` input tensor, so a single compiled
program (~1 min compile) serves both correctness (iters=1) and marginal
HW timing (iters=R vs 1) without recompiling.
"""

import os
import sys

for _p in ("/opt/trn_rl_repo", os.path.expanduser("~/.axon_site/_ro/trn_rl_repo")):
    if os.path.isdir(_p) and _p not in sys.path:
        sys.path.insert(0, _p)
        break

import numpy as np
import ml_dtypes

import concourse.bacc as bacc
import concourse.mybir as mybir
from concourse import masks
from concourse.tile import TileContext
from concourse.bass_utils import run_bass_kernel_spmd

F32 = mybir.dt.float32
BF16 = mybir.dt.bfloat16
FP8 = mybir.dt.float8e4
I32 = mybir.dt.int32
AF = mybir.ActivationFunctionType
ALU = mybir.AluOpType
DR = mybir.MatmulPerfMode.DoubleRow
NEG = np.float32(-1e30)
NPBF = ml_dtypes.bfloat16
NPF8 = ml_dtypes.float8_e4m3

D = 1024
DC = 8  # D // 128
N_CORES = 8

_prog_cache = {}


def build_program(T, stage=5):
    """Build + compile the uniform per-core program for sequence length T."""
    key = (T, stage)
    if key in _prog_cache:
        return _prog_cache[key]

    NQT = T // 256  # query tiles per core
    nc = bacc.Bacc(trn_type="TRN2", target_bir_lowering=False, debug=False,
                   num_devices=N_CORES, dynamic_dma_scratch_size=512)

    xh_in = nc.dram_tensor("xh16", [128, DC, T], BF16, kind="ExternalInput").ap()
    h8k_in = nc.dram_tensor("h8k", [128, DC, T], FP8, kind="ExternalInput").ap()
    l8k_in = nc.dram_tensor("l8k", [128, DC, T], FP8, kind="ExternalInput").ap()
    qh_in = nc.dram_tensor("qh16", [NQT, 128, DC, 128], BF16,
                           kind="ExternalInput").ap()
    q8h_in = nc.dram_tensor("q8h", [NQT, 128, DC, 128], FP8,
                            kind="ExternalInput").ap()
    ql8_in = nc.dram_tensor("ql8", [NQT, 128, DC, 128], FP8,
                            kind="ExternalInput").ap()
    g8h_in = nc.dram_tensor("g8h", [128, T // 128, D], FP8,
                            kind="ExternalInput").ap()
    g8l_in = nc.dram_tensor("g8l", [T // 256, 128, 2, D], FP8,
                            kind="ExternalInput").ap()
    qg_in = nc.dram_tensor("qg", [NQT, 128, D], BF16, kind="ExternalInput").ap()
    qmask_in = nc.dram_tensor("qmask", [128, 256], F32, kind="ExternalInput").ap()
    smask_in = nc.dram_tensor("smask", [128, 256], F32, kind="ExternalInput").ap()
    recip_in = nc.dram_tensor("recipc", [128, NQT], F32, kind="ExternalInput").ap()
    eta_in = nc.dram_tensor("eta", [1, NQT, 128], BF16, kind="ExternalInput").ap()
    biasr_in = nc.dram_tensor("biasr", [1, D], BF16, kind="ExternalInput").ap()
    wg_in = nc.dram_tensor("wg", [128, DC], BF16, kind="ExternalInput").ap()
    sc_in = nc.dram_tensor("sc", [128, 3], F32, kind="ExternalInput").ap()
    it_h = nc.dram_tensor("iters", [1, 1], I32, kind="ExternalInput")
    y_out = nc.dram_tensor("y", [NQT, 128, D], F32, kind="ExternalOutput").ap()

    from contextlib import ExitStack

    with TileContext(nc) as tc, ExitStack() as ctx:
        cpool = ctx.enter_context(tc.tile_pool(name="consts", bufs=1))
        xTp = ctx.enter_context(tc.tile_pool(name="xTp", bufs=1))
        Sp = ctx.enter_context(tc.tile_pool(name="Sp", bufs=1))
        xkp = ctx.enter_context(tc.tile_pool(name="xkp", bufs=3))
        xqp = ctx.enter_context(tc.tile_pool(name="xqp", bufs=1))
        qgp = ctx.enter_context(tc.tile_pool(name="qgp", bufs=2))
        stp = ctx.enter_context(tc.tile_pool(name="stp", bufs=3))
        msgp = ctx.enter_context(tc.tile_pool(name="msgp", bufs=2))
        mixp = ctx.enter_context(tc.tile_pool(name="mixp", bufs=1))
        smallp = ctx.enter_context(tc.tile_pool(name="smallp", bufs=2))
        ps_s = ctx.enter_context(tc.tile_pool(name="ps_s", bufs=2, space="PSUM"))
        ps_t = ctx.enter_context(tc.tile_pool(name="ps_t", bufs=1, space="PSUM"))
        ps_m = ctx.enter_context(tc.tile_pool(name="ps_m", bufs=2, space="PSUM"))
        ps_g = ctx.enter_context(tc.tile_pool(name="ps_g", bufs=1, space="PSUM"))

        qmask = cpool.tile([128, 256], F32)
        nc.sync.dma_start(out=qmask[:], in_=qmask_in[:])
        smask = cpool.tile([128, 256], F32)
        nc.sync.dma_start(out=smask[:], in_=smask_in[:])
        recip = cpool.tile([128, NQT], F32)
        nc.sync.dma_start(out=recip[:], in_=recip_in[:])
        eta = cpool.tile([1, NQT, 128], BF16)
        nc.sync.dma_start(out=eta[:], in_=eta_in[:])
        biasr = cpool.tile([1, D], BF16)
        nc.sync.dma_start(out=biasr[:], in_=biasr_in[:])
        wg = cpool.tile([128, DC], BF16)
        nc.sync.dma_start(out=wg[:], in_=wg_in[:])
        sc = cpool.tile([128, 3], F32)
        nc.sync.dma_start(out=sc[:], in_=sc_in[:])
        ident32 = cpool.tile([128, 128], F32)
        masks.make_identity(nc, ident32[:])

        itreg = nc.alloc_registers("iters_reg", mybir.ALL_ENGINES)
        nc.regs_load(itreg, it_h[0:1, 0:1])
        iters_sv = nc.snap(itreg, donate=True, min_val=0, max_val=1 << 20)

        with tc.For_i(0, iters_sv, name="reps") as _rep:
            # resident x^T encodings, re-DMA'd each rep (honest HBM accounting):
            # bf16(16*xh) for the main pass, fp8(2*xh) and fp8(128*xl) for the
            # DoubleRow correction passes. All scores come out scaled by 256,
            # which the top-8 selection doesn't care about.
            xh = xTp.tile([128, DC, T], BF16, tag="xh", name="xh")
            nc.sync.dma_start(out=xh[:], in_=xh_in[:])
            h8k = xTp.tile([128, DC, T], FP8, tag="h8k", name="h8k")
            nc.sync.dma_start(out=h8k[:], in_=h8k_in[:])
            l8k = xTp.tile([128, DC, T], FP8, tag="l8k", name="l8k")
            nc.sync.dma_start(out=l8k[:], in_=l8k_in[:])
            # resident fp8 hi encoding of x*gain rows (aggregation moving hi)
            g8h = xTp.tile([128, T // 128, D], FP8, tag="g8h", name="g8h")
            nc.sync.dma_start(out=g8h[:], in_=g8h_in[:])

            for t in range(NQT):
                Lc = 2 * t + 2
                Lk = Lc * 128
                qht = xqp.tile([128, DC, 128], BF16, tag="qh", name="qh")
                nc.sync.dma_start(out=qht[:], in_=qh_in[t])
                q8ht = xqp.tile([128, DC, 128], FP8, tag="q8h", name="q8h")
                nc.sync.dma_start(out=q8ht[:], in_=q8h_in[t])
                ql8t = xqp.tile([128, DC, 128], FP8, tag="ql8", name="ql8")
                nc.sync.dma_start(out=ql8t[:], in_=ql8_in[t])
                qgt = qgp.tile([128, D], BF16, tag="qg", name="qg")
                nc.sync.dma_start(out=qgt[:], in_=qg_in[t])

                # ---- scores*256: hi*hi (bf16) + (hi*lo + lo*hi) (fp8 DR) ----
                S = Sp.tile([128, T], F32)
                nblk = (Lk + 511) // 512
                for blk in range(nblk):
                    w = min(512, Lk - blk * 512)
                    lo = blk * 512
                    ps = ps_s.tile([128, 512], F32)
                    for dc in range(DC):
                        nc.tensor.matmul(ps[:, :w], qht[:, dc],
                                         xh[:, dc, lo:lo + w],
                                         start=(dc == 0), stop=False)
                    for qt, xt in ((q8ht, l8k), (ql8t, h8k)):
                        last = qt is ql8t
                        for dp in range(DC // 2):
                            nc.tensor.matmul(
                                ps[:, :w], qt[:, 2 * dp:2 * dp + 2],
                                xt[:, 2 * dp:2 * dp + 2, lo:lo + w],
                                start=False,
                                stop=(last and dp == DC // 2 - 1),
                                perf_mode=DR)
                    plain_w = w if blk < nblk - 1 else w - 256
                    if plain_w > 0:
                        nc.scalar.copy(S[:, lo:lo + plain_w], ps[:, :plain_w])
                    if blk == nblk - 1:
                        nc.vector.tensor_add(S[:, Lk - 256:Lk],
                                             ps[:, w - 256:w], qmask[:])

                if stage <= 2:
                    dbg2 = msgp.tile([128, D], F32, name="dbg2")
                    nc.vector.tensor_copy(dbg2[:], S[:, 0:D])
                    nc.sync.dma_start(out=y_out[t], in_=dbg2[:])
                    continue

                # ---- top-8 threshold -> selection weights in-place ----
                v8 = smallp.tile([128, 8], F32, tag="v8", name="v8")
                nc.vector.max(out=v8[:], in_=S[:, :Lk])
                nc.vector.tensor_scalar(S[:, :Lk], S[:, :Lk], v8[:, 7:8], None,
                                        op0=ALU.is_ge)
                if t == 0:
                    nc.vector.tensor_mul(S[:, :256], S[:, :256], smask[:])

                if stage <= 3:
                    dbg3 = msgp.tile([128, D], F32, name="dbg3")
                    nc.vector.tensor_copy(dbg3[:], S[:, 0:D])
                    nc.sync.dma_start(out=y_out[t], in_=dbg3[:])
                    continue

                # ---- gate (hi-only: |lo.w| ~ 1e-4 abs, negligible) ----
                pg = ps_g.tile([128, 1], F32)
                for dc in range(DC):
                    nc.tensor.matmul(pg[:], qht[:, dc], wg[:, dc:dc + 1],
                                     start=(dc == 0), stop=(dc == DC - 1))
                gate = smallp.tile([128, 1], F32, tag="gate", name="gate")
                nc.scalar.activation(gate[:], pg[:], AF.Sigmoid,
                                     bias=sc[:, 0:1], scale=1.0)
                nc.vector.tensor_mul(gate[:], gate[:], sc[:, 1:2])

                # ---- aggregation (all terms PSUM-scaled x16):
                # pm = 16*sel^T @ (xg_hi + xg_lo) + eta*(16*bias), via fp8
                # DoubleRow over chunk pairs: hi from the resident g8h with a
                # x16-scaled sel^T stationary, lo streamed at x16 encoding
                # with an unscaled sel^T stationary.
                pm = ps_m.tile([128, D], F32)
                for h in (0, 1):
                    nc.tensor.matmul(pm[:, h * 512:(h + 1) * 512],
                                     eta[0:1, t], biasr[0:1, h * 512:(h + 1) * 512],
                                     start=True, stop=False)
                NCP = Lc // 2
                for cp in range(NCP):
                    sT16 = stp.tile([128, 2, 128], FP8, tag="sT16", name="sT16")
                    sT1 = stp.tile([128, 2, 128], FP8, tag="sT1", name="sT1")
                    for j in (0, 1):
                        c = 2 * cp + j
                        pt = ps_t.tile([128, 128], F32, tag="pt", name="pts")
                        nc.tensor.transpose(pt[:], S[:, c * 128:(c + 1) * 128],
                                            ident32[:])
                        nc.scalar.activation(sT16[:, j], pt[:], AF.Copy,
                                             scale=16.0)
                        nc.scalar.copy(sT1[:, j], pt[:])
                    gl = xkp.tile([128, 2, D], FP8, tag="gl", name="gl")
                    nc.sync.dma_start(out=gl[:], in_=g8l_in[cp])
                    for h in (0, 1):
                        hs = slice(h * 512, (h + 1) * 512)
                        nc.tensor.matmul(pm[:, hs], sT16[:],
                                         g8h[:, 2 * cp:2 * cp + 2, hs],
                                         start=False, stop=False, perf_mode=DR)
                        nc.tensor.matmul(pm[:, hs], sT1[:], gl[:, :, hs],
                                         start=False,
                                         stop=(cp == NCP - 1 and h == 1),
                                         perf_mode=DR)

                if stage <= 4:
                    dbg4 = msgp.tile([128, D], F32, name="dbg4")
                    nc.scalar.copy(dbg4[:], pm[:])
                    nc.sync.dma_start(out=y_out[t], in_=dbg4[:])
                    continue

                # ---- tail: z = pm*recip + mix*qg; out = gate * gelu(z) ----
                msg = msgp.tile([128, D], F32)
                nc.scalar.activation(msg[:], pm[:], AF.Copy,
                                     scale=recip[:, t:t + 1])
                mixb = mixp.tile([128, D], F32, tag="mixb", name="mixb")
                nc.scalar.activation(mixb[:], qgt[:], AF.Copy,
                                     scale=sc[:, 2:3])
                nc.vector.tensor_add(msg[:], msg[:], mixb[:])
                nc.scalar.activation(msg[:], msg[:], AF.Gelu)
                nc.vector.tensor_scalar(msg[:], msg[:], gate[:, 0:1], None,
                                        op0=ALU.mult)
                nc.sync.dma_start(out=y_out[t], in_=msg[:])

    nc.compile()
    _prog_cache[key] = nc
    return nc


def host_inputs(xb, p, mix, scale, b_gate, w_gate, gain, bias, T, iters):
    """Per-core input arrays for batch slice xb (T,D) and parity p."""
    NQT = T // 256
    f32 = np.float32
    xb = np.ascontiguousarray(xb, f32)

    xh2 = xb.astype(NPBF).astype(f32)            # (T,D) hi (exact bf16)
    xl2 = (xb - xh2).astype(NPBF).astype(f32)    # (T,D) lo (exact bf16)
    enc = {
        "h16": (16.0 * xh2).astype(NPBF),        # bf16, exact x16
        "h8": (2.0 * xh2).astype(NPF8),          # fp8 e4m3
        "l8": (128.0 * xl2).astype(NPF8),        # fp8 e4m3
    }

    def keyside(a):
        return np.ascontiguousarray(a.reshape(T, DC, 128).transpose(2, 1, 0))

    def queryside(a):
        rows = a.reshape(T // 128, 128, D)[p::2]
        return np.ascontiguousarray(
            rows.reshape(NQT, 128, DC, 128).transpose(0, 3, 2, 1))

    xh = keyside(enc["h16"])
    h8k = keyside(enc["h8"])
    l8k = keyside(enc["l8"])
    qh = queryside(enc["h16"])
    q8h = queryside(enc["h8"])
    ql8 = queryside(enc["l8"])

    r = np.arange(128)
    tri_add = np.where(r[None, :] <= r[:, None], f32(0), NEG).astype(f32)
    tri01 = (r[None, :] <= r[:, None]).astype(f32)
    qmask = np.zeros((128, 256), f32)
    smask = np.zeros((128, 256), f32)
    if p == 0:
        qmask[:, :128] = tri_add
        qmask[:, 128:] = NEG
        smask[:, :128] = tri01
    else:
        qmask[:, 128:] = tri_add
        smask[:, :128] = 1.0
        smask[:, 128:] = tri01

    # counts: count(t, q) = min((2t+p)*128 + q + 1, 8)
    g_row = (2 * np.arange(NQT)[:, None] + p) * 128 + r[None, :]  # (NQT,128)
    counts = np.minimum(g_row + 1, 8).astype(f32)

    # aggregation PSUM carries 16*(sel @ xg) + eta*(16*bias); recip folds
    # the /16 back out so msg = (1-mix)*mean(xg) + bias after the copy.
    recipc = np.ascontiguousarray(((1.0 - mix) / (16.0 * counts)).T)
    eta = np.ascontiguousarray((counts / (1.0 - mix))[None])    # (1, NQT, 128)

    xg = xb * np.asarray(gain, f32)[None, :]                    # (T,D)
    g8h_rows = xg.astype(NPF8)                                  # fp8 hi
    g8l_rows = (16.0 * (xg - g8h_rows.astype(f32))).astype(NPF8)  # fp8 lo x16
    # resident hi: [128=k_in_chunk, T/128 chunks, D]
    g8h = np.ascontiguousarray(g8h_rows.reshape(T // 128, 128, D)
                               .transpose(1, 0, 2))
    # streamed lo chunk-pairs: [T/256, 128, 2, D]
    g8l = np.ascontiguousarray(g8l_rows.reshape(T // 256, 2, 128, D)
                               .transpose(0, 2, 1, 3))
    # query-parity rows of xg (mix*x*gain tail term), bf16
    qg = np.ascontiguousarray(xg.reshape(T // 128, 128, D)[p::2].astype(NPBF))

    # gate rides the bf16(16*xh) stationary, so fold the /16 into w_gate
    wg = np.ascontiguousarray(
        (np.asarray(w_gate, f32) / 16.0).reshape(DC, 128).T)
    sc_arr = np.zeros((128, 3), f32)
    sc_arr[:, 0] = b_gate
    sc_arr[:, 1] = scale
    sc_arr[:, 2] = mix
    return {
        "xh16": xh, "h8k": h8k, "l8k": l8k,
        "qh16": qh, "q8h": q8h, "ql8": ql8,
        "g8h": g8h, "g8l": g8l, "qg": qg,
        "qmask": qmask, "smask": smask,
        "recipc": recipc.astype(f32), "eta": eta.astype(NPBF),
        "biasr": np.ascontiguousarray(
            16.0 * np.asarray(bias, f32)[None, :]).astype(NPBF),
        "wg": wg.astype(NPBF),
        "sc": sc_arr,
        "iters": np.array([[iters]], np.int32),
    }


_inmaps_cache = {}


def run_cores(x, w_gate, b_gate, gain, bias, log_mix, log_scale,
              iters=1, bench=False, stage=5):
    """Run the SPMD program over all 8 cores; returns (B,T,D) output.

    bench=True reuses the (expensive to build) per-core input maps across
    calls, so back-to-back timed launches differ only in the on-device
    `iters` loop count.
    """
    x = np.asarray(x, np.float32)
    B, T, _ = x.shape

    nc = build_program(T, stage=stage)
    ck = (x.shape, stage)
    in_maps = _inmaps_cache.get(ck) if bench else None
    if in_maps is None:
        mix = float(1.0 / (1.0 + np.exp(-np.float64(log_mix))))
        scale = float(np.logaddexp(0.0, np.float64(log_scale)) + 0.01)
        b_gate_f = float(np.asarray(b_gate, np.float64))
        in_maps = []
        for core in range(N_CORES):
            b, p = core >> 1, core & 1
            in_maps.append(host_inputs(x[b % B], p, mix, scale, b_gate_f,
                                       w_gate, gain, bias, T, iters))
        if bench:
            _inmaps_cache[ck] = in_maps
    for m in in_maps:
        m["iters"] = np.array([[iters]], np.int32)
    res = run_bass_kernel_spmd(nc, in_maps, list(range(N_CORES)))
    if bench:
        return None
    out = np.empty((B, T, D), np.float32)
    for core in range(N_CORES):
        b, p = core >> 1, core & 1
        if b >= B:
            continue
        out[b].reshape(T // 128, 128, D)[p::2] = res.results[core]["y"]
    return out


def kernel(x, w_gate, b_gate, gain, bias, log_mix, log_scale, K):
    assert int(K) == 8, "kernel is specialized for K=8"
    return run_cores(x, w_gate, b_gate, gain, bias, log_mix, log_scale)


# revision 11
# speedup vs baseline: 1.8827x; 1.8827x over previous
"""Trainium2 Bass kernel for causal top-K (K=8) similarity message passing.

Math per batch b (reference):
  gate  = sigmoid(x @ w_gate + b_gate)                      (B,T)
  S     = x @ x^T, causal-masked to NEG=-1e30               (B,T,T)
  top-8 per row -> selected neighbour set, count=min(i+1,8)
  msg   = mean of selected x rows
  blend = mix*x + (1-mix)*msg
  out   = gate * gelu(blend*gain + bias) * (softplus(log_scale)+0.01)

Sharding: 8 cores = 4 batches x 2 query-parity shards. Core c handles
batch b=c>>1, parity p=c&1, processing query tiles Qg = 2t+p (t=0..T/256-1)
of 128 rows each. Every core runs a uniform program; all parity
dependence is carried in per-core input data (masks / precomputed tables).

Numerics: scores need enough precision that the top-8 *selection* matches
fp32 (rank-8/9 gaps are O(1) while |score| is O(100); bf16 or f32r alone
flips a few % of rows and blows the 2e-2 budget). x^T is therefore split
host-side into bf16 hi+lo halves and S accumulated as
hi*hi + hi*lo + lo*hi in PSUM -- fp32-grade scores at 3 bf16 matmul
passes (vs 4 passes for native fp32 PE). The aggregation path
(sel^T @ x*gain) is plain bf16: msg is a mean of <=8 unit-scale rows, so
bf16 rounding lands ~1e-3 relative, far inside the 2e-2 budget.

Per query tile t (Lc = 2t+2 key chunks of 128; one wasted fully-masked
chunk for p=0 so both parities run the identical program):
  scores  = sum of 3 split matmuls (PE, bf16) -> PSUM -> SBUF + causal mask
  v8      = max8(scores), tau = v8[:,7]      (DVE top-8 instruction)
  sel     = scores >= tau (0/1), fixed up for tile 0; diagonal gets
            mix*count/(1-mix) added so the blend's mix*x term rides the
            aggregation matmul
  msg     = sel^T-transposed chunks @ (x*gain bf16) accumulated in PSUM,
            plus a rank-1 matmul adding bias*count/(1-mix)
  z       = msg * (1-mix)/count   (per-row scale during PSUM->SBUF copy)
  out     = sigmoid(gate_lin)*scale * gelu(z)

The whole per-core computation sits inside a tc.For_i whose trip count is
read at runtime from the tiny `iters` input tensor, so a single compiled
program (~1 min compile) serves both correctness (iters=1) and marginal
HW timing (iters=R vs 1) without recompiling.
"""

import os
import sys

for _p in ("/opt/trn_rl_repo", os.path.expanduser("~/.axon_site/_ro/trn_rl_repo")):
    if os.path.isdir(_p) and _p not in sys.path:
        sys.path.insert(0, _p)
        break

import numpy as np
import ml_dtypes

# Persistent XLA compile cache: run_bass_kernel_spmd re-traces a fresh jit
# wrapper every call, so without this each launch pays a multi-second XLA
# recompile (the NEFF itself is cached separately by neuronxcc).
import jax as _jax

_jax.config.update("jax_compilation_cache_dir", "/tmp/jax_comp_cache")
_jax.config.update("jax_persistent_cache_min_compile_time_secs", 0.0)
_jax.config.update("jax_persistent_cache_min_entry_size_bytes", 0)

import concourse.bacc as bacc
import concourse.mybir as mybir
from concourse import masks
from concourse.tile import TileContext
from concourse.bass_utils import run_bass_kernel_spmd

F32 = mybir.dt.float32
BF16 = mybir.dt.bfloat16
FP8 = mybir.dt.float8e4
I32 = mybir.dt.int32
AF = mybir.ActivationFunctionType
ALU = mybir.AluOpType
DR = mybir.MatmulPerfMode.DoubleRow
NEG = np.float32(-1e30)
NPBF = ml_dtypes.bfloat16
NPF8 = ml_dtypes.float8_e4m3

D = 1024
DC = 8  # D // 128
N_CORES = 8

_prog_cache = {}


def build_program(T, stage=5):
    """Build + compile the uniform per-core program for sequence length T."""
    key = (T, stage)
    if key in _prog_cache:
        return _prog_cache[key]

    NQT = T // 256  # query tiles per core
    nc = bacc.Bacc(trn_type="TRN2", target_bir_lowering=False, debug=False,
                   num_devices=N_CORES, dynamic_dma_scratch_size=512)

    xh_in = nc.dram_tensor("xh16", [128, DC, T], BF16, kind="ExternalInput").ap()
    h8k_in = nc.dram_tensor("h8k", [128, DC, T], FP8, kind="ExternalInput").ap()
    l8k_in = nc.dram_tensor("l8k", [128, DC, T], FP8, kind="ExternalInput").ap()
    qh_in = nc.dram_tensor("qh16", [NQT, 128, DC, 128], BF16,
                           kind="ExternalInput").ap()
    q8h_in = nc.dram_tensor("q8h", [NQT, 128, DC, 128], FP8,
                            kind="ExternalInput").ap()
    ql8_in = nc.dram_tensor("ql8", [NQT, 128, DC, 128], FP8,
                            kind="ExternalInput").ap()
    g8h_in = nc.dram_tensor("g8h", [128, T // 128, D], FP8,
                            kind="ExternalInput").ap()
    g8l_in = nc.dram_tensor("g8l", [T // 256, 128, 2, D], FP8,
                            kind="ExternalInput").ap()
    qg_in = nc.dram_tensor("qg", [NQT, 128, D], BF16, kind="ExternalInput").ap()
    qmask_in = nc.dram_tensor("qmask", [128, 256], F32, kind="ExternalInput").ap()
    smask_in = nc.dram_tensor("smask", [128, 256], F32, kind="ExternalInput").ap()
    recip_in = nc.dram_tensor("recipc", [128, NQT], F32, kind="ExternalInput").ap()
    eta_in = nc.dram_tensor("eta", [1, NQT, 128], BF16, kind="ExternalInput").ap()
    biasr_in = nc.dram_tensor("biasr", [1, D], BF16, kind="ExternalInput").ap()
    wg_in = nc.dram_tensor("wg", [128, DC], BF16, kind="ExternalInput").ap()
    sc_in = nc.dram_tensor("sc", [128, 3], F32, kind="ExternalInput").ap()
    it_h = nc.dram_tensor("iters", [1, 1], I32, kind="ExternalInput")
    y_out = nc.dram_tensor("y", [NQT, 128, D], F32, kind="ExternalOutput").ap()

    from contextlib import ExitStack

    with TileContext(nc) as tc, ExitStack() as ctx:
        cpool = ctx.enter_context(tc.tile_pool(name="consts", bufs=1))
        xTp = ctx.enter_context(tc.tile_pool(name="xTp", bufs=1))
        Sp = ctx.enter_context(tc.tile_pool(name="Sp", bufs=1))
        xkp = ctx.enter_context(tc.tile_pool(name="xkp", bufs=3))
        xqp = ctx.enter_context(tc.tile_pool(name="xqp", bufs=1))
        qgp = ctx.enter_context(tc.tile_pool(name="qgp", bufs=2))
        stp = ctx.enter_context(tc.tile_pool(name="stp", bufs=3))
        msgp = ctx.enter_context(tc.tile_pool(name="msgp", bufs=2))
        mixp = ctx.enter_context(tc.tile_pool(name="mixp", bufs=1))
        smallp = ctx.enter_context(tc.tile_pool(name="smallp", bufs=2))
        ps_s = ctx.enter_context(tc.tile_pool(name="ps_s", bufs=2, space="PSUM"))
        ps_t = ctx.enter_context(tc.tile_pool(name="ps_t", bufs=1, space="PSUM"))
        ps_m = ctx.enter_context(tc.tile_pool(name="ps_m", bufs=2, space="PSUM"))
        ps_g = ctx.enter_context(tc.tile_pool(name="ps_g", bufs=1, space="PSUM"))

        qmask = cpool.tile([128, 256], F32)
        nc.sync.dma_start(out=qmask[:], in_=qmask_in[:])
        smask = cpool.tile([128, 256], F32)
        nc.sync.dma_start(out=smask[:], in_=smask_in[:])
        recip = cpool.tile([128, NQT], F32)
        nc.sync.dma_start(out=recip[:], in_=recip_in[:])
        eta = cpool.tile([1, NQT, 128], BF16)
        nc.sync.dma_start(out=eta[:], in_=eta_in[:])
        biasr = cpool.tile([1, D], BF16)
        nc.sync.dma_start(out=biasr[:], in_=biasr_in[:])
        wg = cpool.tile([128, DC], BF16)
        nc.sync.dma_start(out=wg[:], in_=wg_in[:])
        sc = cpool.tile([128, 3], F32)
        nc.sync.dma_start(out=sc[:], in_=sc_in[:])
        ident32 = cpool.tile([128, 128], F32)
        masks.make_identity(nc, ident32[:])

        itreg = nc.alloc_registers("iters_reg", mybir.ALL_ENGINES)
        nc.regs_load(itreg, it_h[0:1, 0:1])
        iters_sv = nc.snap(itreg, donate=True, min_val=0, max_val=1 << 20)

        with tc.For_i(0, iters_sv, name="reps") as _rep:
            # resident x^T encodings, re-DMA'd each rep (honest HBM accounting):
            # bf16(16*xh) for the main pass, fp8(2*xh) and fp8(128*xl) for the
            # DoubleRow correction passes. All scores come out scaled by 256,
            # which the top-8 selection doesn't care about.
            xh = xTp.tile([128, DC, T], BF16, tag="xh", name="xh")
            nc.sync.dma_start(out=xh[:], in_=xh_in[:])
            h8k = xTp.tile([128, DC, T], FP8, tag="h8k", name="h8k")
            nc.sync.dma_start(out=h8k[:], in_=h8k_in[:])
            l8k = xTp.tile([128, DC, T], FP8, tag="l8k", name="l8k")
            nc.sync.dma_start(out=l8k[:], in_=l8k_in[:])
            # resident fp8 hi encoding of x*gain rows (aggregation moving hi)
            g8h = xTp.tile([128, T // 128, D], FP8, tag="g8h", name="g8h")
            nc.sync.dma_start(out=g8h[:], in_=g8h_in[:])

            for t in range(NQT):
                Lc = 2 * t + 2
                Lk = Lc * 128
                qht = xqp.tile([128, DC, 128], BF16, tag="qh", name="qh")
                nc.sync.dma_start(out=qht[:], in_=qh_in[t])
                q8ht = xqp.tile([128, DC, 128], FP8, tag="q8h", name="q8h")
                nc.sync.dma_start(out=q8ht[:], in_=q8h_in[t])
                ql8t = xqp.tile([128, DC, 128], FP8, tag="ql8", name="ql8")
                nc.sync.dma_start(out=ql8t[:], in_=ql8_in[t])
                qgt = qgp.tile([128, D], BF16, tag="qg", name="qg")
                nc.sync.dma_start(out=qgt[:], in_=qg_in[t])

                # ---- scores*256: hi*hi (bf16) + (hi*lo + lo*hi) (fp8 DR) ----
                S = Sp.tile([128, T], F32)
                nblk = (Lk + 511) // 512
                for blk in range(nblk):
                    w = min(512, Lk - blk * 512)
                    lo = blk * 512
                    ps = ps_s.tile([128, 512], F32)
                    for dc in range(DC):
                        nc.tensor.matmul(ps[:, :w], qht[:, dc],
                                         xh[:, dc, lo:lo + w],
                                         start=(dc == 0), stop=False)
                    for qt, xt in ((q8ht, l8k), (ql8t, h8k)):
                        last = qt is ql8t
                        for dp in range(DC // 2):
                            nc.tensor.matmul(
                                ps[:, :w], qt[:, 2 * dp:2 * dp + 2],
                                xt[:, 2 * dp:2 * dp + 2, lo:lo + w],
                                start=False,
                                stop=(last and dp == DC // 2 - 1),
                                perf_mode=DR)
                    plain_w = w if blk < nblk - 1 else w - 256
                    if plain_w > 0:
                        nc.scalar.copy(S[:, lo:lo + plain_w], ps[:, :plain_w])
                    if blk == nblk - 1:
                        nc.vector.tensor_add(S[:, Lk - 256:Lk],
                                             ps[:, w - 256:w], qmask[:])

                if stage <= 2:
                    dbg2 = msgp.tile([128, D], F32, name="dbg2")
                    nc.vector.tensor_copy(dbg2[:], S[:, 0:D])
                    nc.sync.dma_start(out=y_out[t], in_=dbg2[:])
                    continue

                # ---- top-8 threshold -> selection weights in-place ----
                v8 = smallp.tile([128, 8], F32, tag="v8", name="v8")
                nc.vector.max(out=v8[:], in_=S[:, :Lk])
                nc.vector.tensor_scalar(S[:, :Lk], S[:, :Lk], v8[:, 7:8], None,
                                        op0=ALU.is_ge)
                if t == 0:
                    nc.vector.tensor_mul(S[:, :256], S[:, :256], smask[:])

                if stage <= 3:
                    dbg3 = msgp.tile([128, D], F32, name="dbg3")
                    nc.vector.tensor_copy(dbg3[:], S[:, 0:D])
                    nc.sync.dma_start(out=y_out[t], in_=dbg3[:])
                    continue

                # ---- gate (hi-only: |lo.w| ~ 1e-4 abs, negligible) ----
                pg = ps_g.tile([128, 1], F32)
                for dc in range(DC):
                    nc.tensor.matmul(pg[:], qht[:, dc], wg[:, dc:dc + 1],
                                     start=(dc == 0), stop=(dc == DC - 1))
                gate = smallp.tile([128, 1], F32, tag="gate", name="gate")
                nc.scalar.activation(gate[:], pg[:], AF.Sigmoid,
                                     bias=sc[:, 0:1], scale=1.0)
                nc.vector.tensor_mul(gate[:], gate[:], sc[:, 1:2])

                # ---- aggregation (all terms PSUM-scaled x16):
                # pm = 16*sel^T @ (xg_hi + xg_lo) + eta*(16*bias), via fp8
                # DoubleRow over chunk pairs: hi from the resident g8h with a
                # x16-scaled sel^T stationary, lo streamed at x16 encoding
                # with an unscaled sel^T stationary.
                pm = ps_m.tile([128, D], F32)
                for h in (0, 1):
                    nc.tensor.matmul(pm[:, h * 512:(h + 1) * 512],
                                     eta[0:1, t], biasr[0:1, h * 512:(h + 1) * 512],
                                     start=True, stop=False)
                NCP = Lc // 2
                for cp in range(NCP):
                    sT16 = stp.tile([128, 2, 128], FP8, tag="sT16", name="sT16")
                    sT1 = stp.tile([128, 2, 128], FP8, tag="sT1", name="sT1")
                    for j in (0, 1):
                        c = 2 * cp + j
                        pt = ps_t.tile([128, 128], F32, tag="pt", name="pts")
                        nc.tensor.transpose(pt[:], S[:, c * 128:(c + 1) * 128],
                                            ident32[:])
                        nc.scalar.activation(sT16[:, j], pt[:], AF.Copy,
                                             scale=16.0)
                        nc.scalar.copy(sT1[:, j], pt[:])
                    gl = xkp.tile([128, 2, D], FP8, tag="gl", name="gl")
                    nc.sync.dma_start(out=gl[:], in_=g8l_in[cp])
                    for h in (0, 1):
                        hs = slice(h * 512, (h + 1) * 512)
                        nc.tensor.matmul(pm[:, hs], sT16[:],
                                         g8h[:, 2 * cp:2 * cp + 2, hs],
                                         start=False, stop=False, perf_mode=DR)
                        nc.tensor.matmul(pm[:, hs], sT1[:], gl[:, :, hs],
                                         start=False,
                                         stop=(cp == NCP - 1 and h == 1),
                                         perf_mode=DR)

                if stage <= 4:
                    dbg4 = msgp.tile([128, D], F32, name="dbg4")
                    nc.scalar.copy(dbg4[:], pm[:])
                    nc.sync.dma_start(out=y_out[t], in_=dbg4[:])
                    continue

                # ---- tail: z = pm*recip + mix*qg; out = gate * gelu(z) ----
                msg = msgp.tile([128, D], F32)
                nc.scalar.activation(msg[:], pm[:], AF.Copy,
                                     scale=recip[:, t:t + 1])
                mixb = mixp.tile([128, D], F32, tag="mixb", name="mixb")
                nc.scalar.activation(mixb[:], qgt[:], AF.Copy,
                                     scale=sc[:, 2:3])
                nc.vector.tensor_add(msg[:], msg[:], mixb[:])
                nc.scalar.activation(msg[:], msg[:], AF.Gelu)
                nc.vector.tensor_scalar(msg[:], msg[:], gate[:, 0:1], None,
                                        op0=ALU.mult)
                nc.sync.dma_start(out=y_out[t], in_=msg[:])

    nc.compile()
    _prog_cache[key] = nc
    return nc


def host_inputs(xb, p, mix, scale, b_gate, w_gate, gain, bias, T, iters):
    """Per-core input arrays for batch slice xb (T,D) and parity p."""
    NQT = T // 256
    f32 = np.float32
    xb = np.ascontiguousarray(xb, f32)

    xh2 = xb.astype(NPBF).astype(f32)            # (T,D) hi (exact bf16)
    xl2 = (xb - xh2).astype(NPBF).astype(f32)    # (T,D) lo (exact bf16)
    enc = {
        "h16": (16.0 * xh2).astype(NPBF),        # bf16, exact x16
        "h8": (2.0 * xh2).astype(NPF8),          # fp8 e4m3
        "l8": (128.0 * xl2).astype(NPF8),        # fp8 e4m3
    }

    def keyside(a):
        return np.ascontiguousarray(a.reshape(T, DC, 128).transpose(2, 1, 0))

    def queryside(a):
        rows = a.reshape(T // 128, 128, D)[p::2]
        return np.ascontiguousarray(
            rows.reshape(NQT, 128, DC, 128).transpose(0, 3, 2, 1))

    xh = keyside(enc["h16"])
    h8k = keyside(enc["h8"])
    l8k = keyside(enc["l8"])
    qh = queryside(enc["h16"])
    q8h = queryside(enc["h8"])
    ql8 = queryside(enc["l8"])

    r = np.arange(128)
    tri_add = np.where(r[None, :] <= r[:, None], f32(0), NEG).astype(f32)
    tri01 = (r[None, :] <= r[:, None]).astype(f32)
    qmask = np.zeros((128, 256), f32)
    smask = np.zeros((128, 256), f32)
    if p == 0:
        qmask[:, :128] = tri_add
        qmask[:, 128:] = NEG
        smask[:, :128] = tri01
    else:
        qmask[:, 128:] = tri_add
        smask[:, :128] = 1.0
        smask[:, 128:] = tri01

    # counts: count(t, q) = min((2t+p)*128 + q + 1, 8)
    g_row = (2 * np.arange(NQT)[:, None] + p) * 128 + r[None, :]  # (NQT,128)
    counts = np.minimum(g_row + 1, 8).astype(f32)

    # aggregation PSUM carries 16*(sel @ xg) + eta*(16*bias); recip folds
    # the /16 back out so msg = (1-mix)*mean(xg) + bias after the copy.
    recipc = np.ascontiguousarray(((1.0 - mix) / (16.0 * counts)).T)
    eta = np.ascontiguousarray((counts / (1.0 - mix))[None])    # (1, NQT, 128)

    xg = xb * np.asarray(gain, f32)[None, :]                    # (T,D)
    g8h_rows = xg.astype(NPF8)                                  # fp8 hi
    g8l_rows = (16.0 * (xg - g8h_rows.astype(f32))).astype(NPF8)  # fp8 lo x16
    # resident hi: [128=k_in_chunk, T/128 chunks, D]
    g8h = np.ascontiguousarray(g8h_rows.reshape(T // 128, 128, D)
                               .transpose(1, 0, 2))
    # streamed lo chunk-pairs: [T/256, 128, 2, D]
    g8l = np.ascontiguousarray(g8l_rows.reshape(T // 256, 2, 128, D)
                               .transpose(0, 2, 1, 3))
    # query-parity rows of xg (mix*x*gain tail term), bf16
    qg = np.ascontiguousarray(xg.reshape(T // 128, 128, D)[p::2].astype(NPBF))

    # gate rides the bf16(16*xh) stationary, so fold the /16 into w_gate
    wg = np.ascontiguousarray(
        (np.asarray(w_gate, f32) / 16.0).reshape(DC, 128).T)
    sc_arr = np.zeros((128, 3), f32)
    sc_arr[:, 0] = b_gate
    sc_arr[:, 1] = scale
    sc_arr[:, 2] = mix
    return {
        "xh16": xh, "h8k": h8k, "l8k": l8k,
        "qh16": qh, "q8h": q8h, "ql8": ql8,
        "g8h": g8h, "g8l": g8l, "qg": qg,
        "qmask": qmask, "smask": smask,
        "recipc": recipc.astype(f32), "eta": eta.astype(NPBF),
        "biasr": np.ascontiguousarray(
            16.0 * np.asarray(bias, f32)[None, :]).astype(NPBF),
        "wg": wg.astype(NPBF),
        "sc": sc_arr,
        "iters": np.array([[iters]], np.int32),
    }


_inmaps_cache = {}


def run_cores(x, w_gate, b_gate, gain, bias, log_mix, log_scale,
              iters=1, bench=False, stage=5):
    """Run the SPMD program over all 8 cores; returns (B,T,D) output.

    bench=True reuses the (expensive to build) per-core input maps across
    calls, so back-to-back timed launches differ only in the on-device
    `iters` loop count.
    """
    x = np.asarray(x, np.float32)
    B, T, _ = x.shape

    nc = build_program(T, stage=stage)
    ck = (x.shape, stage)
    in_maps = _inmaps_cache.get(ck) if bench else None
    if in_maps is None:
        mix = float(1.0 / (1.0 + np.exp(-np.float64(log_mix))))
        scale = float(np.logaddexp(0.0, np.float64(log_scale)) + 0.01)
        b_gate_f = float(np.asarray(b_gate, np.float64))
        in_maps = []
        for core in range(N_CORES):
            b, p = core >> 1, core & 1
            in_maps.append(host_inputs(x[b % B], p, mix, scale, b_gate_f,
                                       w_gate, gain, bias, T, iters))
        if bench:
            _inmaps_cache[ck] = in_maps
    for m in in_maps:
        m["iters"] = np.array([[iters]], np.int32)
    res = run_bass_kernel_spmd(nc, in_maps, list(range(N_CORES)))
    if bench:
        return None
    out = np.empty((B, T, D), np.float32)
    for core in range(N_CORES):
        b, p = core >> 1, core & 1
        if b >= B:
            continue
        out[b].reshape(T // 128, 128, D)[p::2] = res.results[core]["y"]
    return out


def kernel(x, w_gate, b_gate, gain, bias, log_mix, log_scale, K):
    assert int(K) == 8, "kernel is specialized for K=8"
    return run_cores(x, w_gate, b_gate, gain, bias, log_mix, log_scale)
